# revision 1
# baseline (speedup 1.0000x reference)
"""Chamfer distance kernel for Trainium2 (8 NeuronCores, SPMD).

Strategy
--------
x is sharded across 8 cores (2048 rows each); y (16384 points) is replicated.
Each core computes its [2048, 16384] block of the squared-distance matrix
d_ij = |x_i - y_j|^2 via K=13 bf16 matmuls and reduces it on the fly:

The ScalarE evacuates each [128, 2048] PSUM group as NEGATED fp16 (-d) so
that both min directions become maxes on cheap engines:
 * row mins (dist1): per x-tile, 2x-mode fp16 tensor_tensor(max) ops fold
   the 8 groups into one [128, 2048] buffer, then a single 1x tensor_reduce
   per x-tile finishes the row (8x fewer 1x reduces than reducing per group)
 * column mins (dist2): tensor_tensor(max) folds each group into a resident
   [128, 16384] fp16 buffer; PE transposes of its 128-wide chunks plus 3D-AP
   DVE reduces finish the cross-partition fold, overlapping the main loop.

Numerical trick: y is kd-sorted into 32 spatially compact tiles of 512; both
point sets are translated by the tile centroid before augmentation
(d is translation invariant), and each translated coordinate is split into
bf16 hi+lo parts. All PE products are then exact and small, so the K=13
bf16 matmul reproduces d to ~f32 quality despite the catastrophic
cancellation in x2+y2-2xy (measured end-to-end rel err ~8e-5).

The host finishes with a trivial O(N) reduction: sum of row mins, min of the
8 per-core column-min vectors, mean.
"""
import sys

sys.path.insert(0, "/opt/trn_rl_repo")

import numpy as np
import ml_dtypes

import concourse.bass as bass
import concourse.tile as tile
from concourse import bacc, mybir
from concourse import bass_utils
from concourse.bass_isa import ReduceOp

BF16 = ml_dtypes.bfloat16

# Problem geometry (hardcoded per the task contract).
N = 16384          # x points
M = 16384          # y points
D = 3
NCORES = 8
XSHARD = N // NCORES        # 2048 x rows per core
P = 128                     # partitions
YTILE = 512                 # translation granularity == matmul moving width
NYT = M // YTILE            # 32 y tiles
YGRP = 2048                 # PSUM group width (4 banks)
NYG = M // YGRP             # 8 groups
NXT = XSHARD // P           # 16 x tiles per core
K = 13                      # augmented contraction depth
VPER = 3                    # XT variants packed per 128-partition page
NPAGES = (NYT + VPER - 1) // VPER   # 11
NCHUNK = M // P             # 128 column chunks of C
INF = 3.0e38
GP_PARITY = 1   # yg slices with this parity run their C-update on GPSIMD


def _bf16_pair(a):
    """Split float64 array into (hi, lo) bf16 parts."""
    hi = a.astype(BF16)
    lo = (a - hi.astype(np.float64)).astype(BF16)
    return hi, lo


def kd_sort(y, n_tiles):
    """Recursive median splits -> permutation grouping y into n_tiles
    spatially compact tiles (n_tiles must be a power of two)."""
    groups = [np.arange(len(y))]
    while len(groups) < n_tiles:
        nxt = []
        for g in groups:
            pts = y[g]
            dim = int(np.argmax(pts.max(0) - pts.min(0)))
            order = np.argsort(pts[:, dim], kind="stable")
            half = len(g) // 2
            nxt.append(g[order[:half]])
            nxt.append(g[order[half:]])
        groups = nxt
    return np.concatenate(groups)


def build_nc(n_xt=NXT, n_yg=NYG):
    """Build the SPMD Bass program (same NEFF on all cores)."""
    n_yt = n_yg * (YGRP // YTILE)
    n_pages = (n_yt + VPER - 1) // VPER
    m = n_yt * YTILE
    n_chunk = m // P
    xshard = n_xt * P
    xt_cols = n_pages * xshard
    out_w = n_xt + n_chunk

    nc = bacc.Bacc("TRN2", target_bir_lowering=False, debug=False,
                   num_devices=NCORES)
    xt_d = nc.dram_tensor("xt", [P, xt_cols], mybir.dt.bfloat16,
                          kind="ExternalInput")
    yt_d = nc.dram_tensor("yt", [64 + K, m], mybir.dt.bfloat16,
                          kind="ExternalInput")
    id_d = nc.dram_tensor("ident", [P, P], mybir.dt.float16,
                          kind="ExternalInput")
    out_d = nc.dram_tensor("out", [P, out_w], mybir.dt.float32,
                           kind="ExternalOutput")

    with tile.TileContext(nc) as tc:
        with (
            tc.tile_pool(name="const", bufs=1) as cpool,
            tc.tile_pool(name="spool", bufs=3) as spool,
            tc.tile_pool(name="ps", bufs=2, space="PSUM") as pspool,
        ):
            xt_t = cpool.tile([P, xt_cols], mybir.dt.bfloat16)
            yt_t = cpool.tile([64 + K, m], mybir.dt.bfloat16)
            id_t = cpool.tile([P, P], mybir.dt.float16)
            c_t = cpool.tile([P, m], mybir.dt.float16)
            out_t = cpool.tile([P, out_w], mybir.dt.float32)
            nc.sync.dma_start(xt_t[:], xt_d.ap())
            nc.sync.dma_start(yt_t[:], yt_d.ap())
            nc.sync.dma_start(id_t[:], id_d.ap())

            # xt outer: fold each x-tile's 8 groups into one fp16 buffer B
            # with 2x tensor_tensor(max) ops, then a single 1x reduce per
            # x-tile -- much cheaper than a 1x reduce per group. All values
            # negated (-d) so mins become GPSIMD/DVE-supported maxes.
            for xt in range(n_xt):
                b_t = spool.tile([P, YGRP], mybir.dt.float16, tag="b")
                for yg in range(n_yg):
                    ps = pspool.tile([P, YGRP], mybir.dt.float32, tag="d")
                    for j4 in range(YGRP // YTILE):
                        j = yg * (YGRP // YTILE) + j4
                        page, slot = divmod(j, VPER)
                        lhsT = xt_t[slot * 32: slot * 32 + K,
                                    page * xshard + xt * P:
                                    page * xshard + (xt + 1) * P]
                        nc.tensor.matmul(
                            ps[:, j4 * YTILE:(j4 + 1) * YTILE],
                            lhsT,
                            yt_t[slot * 32: slot * 32 + K,
                                 j * YTILE:(j + 1) * YTILE],
                            start=True, stop=True,
                        )
                    s_t = spool.tile([P, YGRP], mybir.dt.float16, tag="s")
                    nc.scalar.mul(s_t[:], ps[:], -1.0)
                    if yg == 0:
                        nc.vector.tensor_copy(b_t[:], s_t[:])
                    else:
                        nc.vector.tensor_tensor(b_t[:], s_t[:], b_t[:],
                                                mybir.AluOpType.max)
                    cs = c_t[:, yg * YGRP:(yg + 1) * YGRP]
                    if xt == 0:
                        nc.vector.tensor_copy(cs, s_t[:])
                    else:
                        nc.vector.tensor_tensor(cs, s_t[:], cs,
                                                mybir.AluOpType.max)
                # out[:, xt] = max_j(-d) = -rowmin for this x-tile
                nc.vector.tensor_reduce(
                    out_t[:, xt:xt + 1], b_t[:],
                    axis=mybir.AxisListType.X, op=mybir.AluOpType.max,
                )

            # Fold C (holding -d) across partitions: PE-transpose 128-wide
            # chunks into PSUM, then one 3D-AP reduce(max) per 4 chunks.
            # Cheap on both engines and overlaps the main loop per slice.
            for t4 in range((n_chunk + 3) // 4):
                nt = min(4, n_chunk - t4 * 4)
                pt = pspool.tile([P, nt * P], mybir.dt.float16, tag="d")
                for kk in range(nt):
                    t = t4 * 4 + kk
                    nc.tensor.transpose(
                        pt[:, kk * P:(kk + 1) * P],
                        c_t[:, t * P:(t + 1) * P],
                        id_t[:],
                    )
                nc.vector.tensor_reduce(
                    out_t[:, n_xt + t4 * 4: n_xt + t4 * 4 + nt],
                    pt[:].rearrange("p (a b) -> p a b", b=P),
                    axis=mybir.AxisListType.X, op=mybir.AluOpType.max,
                )

            nc.sync.dma_start(out_d.ap(), out_t[:])

    nc.compile()
    return nc


def prep_inputs(x, y, n_xt=NXT, n_yg=NYG):
    """Host-side: kd-sort y, per-tile translate+augment+bf16-split, pack."""
    n_yt = n_yg * (YGRP // YTILE)
    n_pages = (n_yt + VPER - 1) // VPER
    m = n_yt * YTILE
    xshard = n_xt * P
    ncores = x.shape[0] // xshard

    perm = kd_sort(y, n_yt)
    ys = y[perm].astype(np.float64)

    yt = np.zeros((K, m), dtype=BF16)
    xts = [np.zeros((P, n_pages * xshard), dtype=BF16) for _ in range(ncores)]
    x64 = x.astype(np.float64)

    for j in range(n_yt):
        sl = slice(j * YTILE, (j + 1) * YTILE)
        yb = ys[sl]
        c = yb.mean(0)
        yp = yb - c
        yh, yl = _bf16_pair(yp)
        y2h, y2l = _bf16_pair((yp ** 2).sum(1))
        # rhs rows: yh(3), yl(3), yh(3), 1, 1, y2h, y2l
        yt[0:3, sl] = yh.T
        yt[3:6, sl] = yl.T
        yt[6:9, sl] = yh.T
        yt[9, sl] = BF16(1.0)
        yt[10, sl] = BF16(1.0)
        yt[11, sl] = y2h
        yt[12, sl] = y2l

        page, slot = divmod(j, VPER)
        xp_all = x64 - c
        x2_all = (xp_all ** 2).sum(1)
        for cidx in range(ncores):
            xp = xp_all[cidx * xshard:(cidx + 1) * xshard]
            x2 = x2_all[cidx * xshard:(cidx + 1) * xshard]
            xh, xl = _bf16_pair(xp)
            m2h = (-2.0 * xh.astype(np.float64)).astype(BF16)
            m2l = (-2.0 * xl.astype(np.float64)).astype(BF16)
            x2h, x2l = _bf16_pair(x2)
            blk = np.zeros((K, xshard), dtype=BF16)
            # lhsT rows paired with rhs rows above:
            blk[0:3] = m2h.T          # . yh
            blk[3:6] = m2h.T          # . yl
            blk[6:9] = m2l.T          # . yh
            blk[9] = x2h              # . 1
            blk[10] = x2l             # . 1
            blk[11] = BF16(1.0)       # . y2h
            blk[12] = BF16(1.0)       # . y2l
            xts[cidx][slot * 32: slot * 32 + K,
                      page * xshard:(page + 1) * xshard] = blk

    ident = np.eye(P, dtype=np.float16)
    yt_rep = np.zeros((64 + K, m), dtype=BF16)
    for s in range(VPER):
        yt_rep[s * 32: s * 32 + K] = yt
    in_maps = [
        {"xt": xts[cidx], "yt": yt_rep, "ident": ident}
        for cidx in range(ncores)
    ]
    return in_maps


def postprocess(results, n_xt=NXT, n_yg=NYG):
    n_yt = n_yg * (YGRP // YTILE)
    m = n_yt * YTILE
    n_chunk = m // P
    d1_sum = 0.0
    d2 = np.full((P, n_chunk), np.inf, np.float64)
    for res in results:
        out = res["out"].astype(np.float64)
        d1_sum += np.maximum(-out[:, :n_xt], 0.0).sum()
        d2 = np.minimum(d2, -out[:, n_xt:])
    d2_sum = np.maximum(d2, 0.0).sum()
    n_x = n_xt * P * len(results)
    return (d1_sum + d2_sum) / (n_x + m)


_NC_CACHE = {}


def kernel(x, y):
    x = np.asarray(x, np.float32)
    y = np.asarray(y, np.float32)
    key = "full"
    if key not in _NC_CACHE:
        _NC_CACHE[key] = build_nc()
    nc = _NC_CACHE[key]
    in_maps = prep_inputs(x, y)
    res = bass_utils.run_bass_kernel_spmd(nc, in_maps,
                                          core_ids=list(range(NCORES)))
    val = postprocess(res.results)
    return np.array(val, dtype=np.float32)


if __name__ == "__main__":
    np.random.seed(0)
    x = np.random.randn(N, D).astype(np.float32)
    y = np.random.randn(M, D).astype(np.float32)
    print("kernel:", kernel(x, y))



# revision 6
# speedup vs baseline: 9.8269x; 9.8269x over previous
"""Chamfer distance kernel for Trainium2 (8 NeuronCores, SPMD).

Strategy: pruned nearest-neighbour evaluation.
------------------------------------------------
Both point sets are kd-sorted into 128 spatially compact tiles of 128
points.  On the host (during input prep) rigorous triangle-inequality
bounds select, for every stationary tile, the small set of moving blocks
that can possibly contain a nearest neighbour of any of its points:
  ub(p) = exact min distance from p to the points of its 2 nearest blocks
  lb(p,B) = squared distance from p to block B's AABB
  block B is a candidate for tile T iff any p in T has lb(p,B) <= ub(p).
On this data that keeps ~3.3 candidate blocks per tile (~2.6% of the
dense 16384x16384 distance matrix) while provably containing every true
nearest neighbour, so the result is exact up to arithmetic rounding.

Two passes: pass A (stationary = x tiles) produces the row mins (dist1),
pass B (stationary = y tiles) produces the column mins (dist2) -- no
cross-partition reduction and no inter-core communication is needed;
the host just gathers per-tile min vectors.

SPMD layout: all 8 cores run one NEFF.  Tiles are snake-dealt to cores
by candidate count and sorted within each core, so group g has the same
(max-padded) width on every core; padding duplicates a real candidate
block (harmless under max).  Data differences live entirely in the
per-core input tensors; instruction offsets are structure-only.

Per group the whole stationary tile shares one translation (its own
centroid), so the PE loads one [13,128] stationary operand per group and
streams the group's candidate blocks as 512-wide moving chunks (one
PSUM bank each), computing NEGATED squared distances
  -d = 2x.y - x2 - y2
via a K=13 bf16 contraction (hi/lo-split coordinates keep the products
exact enough for ~1e-4 relative error).  One DVE tensor_reduce(max) per
group then yields -min d for each stationary point directly from PSUM.
"""
import sys

sys.path.insert(0, "/opt/trn_rl_repo")

import numpy as np
import ml_dtypes

import concourse.bass as bass
import concourse.tile as tile
from concourse import bacc, mybir
from concourse import bass_utils

BF16 = ml_dtypes.bfloat16

N = 16384
M = 16384
D = 3
NCORES = 8
P = 128                 # partition dim == stationary tile size
W = 128                 # moving block size
NTILE = N // P          # 128 tiles each side
K = 13                  # augmented contraction depth
KMAX = 16               # max blocks per PSUM group (4 banks of fp32)
BANK = 512              # fp32 per PSUM bank == matmul chunk width


def _bf16_pair(a):
    hi = a.astype(BF16)
    lo = (a - hi.astype(np.float64)).astype(BF16)
    return hi, lo


def kd_sort(pts, n_tiles):
    groups = [np.arange(len(pts))]
    while len(groups) < n_tiles:
        nxt = []
        for g in groups:
            p = pts[g]
            dim = int(np.argmax(p.max(0) - p.min(0)))
            order = np.argsort(p[:, dim], kind="stable")
            half = len(g) // 2
            nxt.append(g[order[:half]])
            nxt.append(g[order[half:]])
        groups = nxt
    return np.concatenate(groups)


def _candidates(stat, mov):
    """Per stationary 128-tile: candidate moving 128-blocks (rigorous)."""
    nmv = len(mov) // W
    mv = mov.reshape(nmv, W, 3)
    lo, hi = mv.min(1), mv.max(1)
    cm = mv.mean(1)
    d2c = ((stat[:, None, :] - cm[None]) ** 2).sum(-1)
    ub = np.full(len(stat), np.inf)
    near2 = np.argsort(d2c, 1)[:, :2]
    for col in range(2):
        near = near2[:, col]
        for j in range(nmv):
            sel = near == j
            if sel.any():
                d = ((stat[sel][:, None, :] - mv[j][None]) ** 2).sum(-1).min(1)
                ub[sel] = np.minimum(ub[sel], d)
    dx = np.maximum(np.maximum(lo[None] - stat[:, None],
                               stat[:, None] - hi[None]), 0.0)
    lb = (dx ** 2).sum(-1)
    needed = lb <= ub[:, None] * (1 + 1e-9)
    pair = needed.reshape(-1, P, nmv).any(1)
    return [np.flatnonzero(pair[t]).tolist() for t in range(pair.shape[0])]


def _deal(cands):
    """Snake-deal tiles to cores by k desc; split k>KMAX; sort within core.
    Returns per-core group lists [(tile_id, [block ids])] (padded with None
    groups so counts match) plus the shared per-position widths."""
    groups = []
    for t, cl in enumerate(cands):
        for s in range(0, len(cl), KMAX):
            groups.append((t, cl[s:s + KMAX]))
    groups.sort(key=lambda g: -len(g[1]))
    cores = [[] for _ in range(NCORES)]
    for n, g in enumerate(groups):
        c = n % (2 * NCORES)
        c = c if c < NCORES else 2 * NCORES - 1 - c
        cores[c].append(g)
    ng = max(len(c) for c in cores)
    for c in cores:
        while len(c) < ng:
            c.append((None, []))
        c.sort(key=lambda g: -len(g[1]))
    widths = [max(1, max(len(cores[c][g][1]) for c in range(NCORES)))
              for g in range(ng)]
    return cores, widths


def _build_structure(x, y):
    xp = kd_sort(x, NTILE)
    yp = kd_sort(y, NTILE)
    xs, ys = x[xp], y[yp]
    candA = _candidates(xs, ys)     # x tiles -> y blocks
    candB = _candidates(ys, xs)     # y tiles -> x blocks
    coresA, widthsA = _deal(candA)
    coresB, widthsB = _deal(candB)
    return dict(xp=xp, yp=yp, xs=xs, ys=ys,
                coresA=coresA, widthsA=widthsA,
                coresB=coresB, widthsB=widthsB)


def build_nc(widthsA, widthsB):
    ngA, ngB = len(widthsA), len(widthsB)
    wA = sum(widthsA) * W
    wB = sum(widthsB) * W

    nc = bacc.Bacc("TRN2", target_bir_lowering=False, debug=False,
                   num_devices=NCORES)
    al_d = nc.dram_tensor("al", [K, ngA * P], mybir.dt.bfloat16,
                          kind="ExternalInput")
    ar_d = nc.dram_tensor("ar", [K, wA], mybir.dt.bfloat16,
                          kind="ExternalInput")
    bl_d = nc.dram_tensor("bl", [K, ngB * P], mybir.dt.bfloat16,
                          kind="ExternalInput")
    br_d = nc.dram_tensor("br", [K, wB], mybir.dt.bfloat16,
                          kind="ExternalInput")
    out_d = nc.dram_tensor("out", [P, ngA + ngB], mybir.dt.float32,
                           kind="ExternalOutput")

    with tile.TileContext(nc) as tc:
        with (
            tc.tile_pool(name="const", bufs=1) as cpool,
            tc.tile_pool(name="ps", bufs=2, space="PSUM") as pspool,
        ):
            al_t = cpool.tile([K, ngA * P], mybir.dt.bfloat16)
            ar_t = cpool.tile([K, wA], mybir.dt.bfloat16)
            bl_t = cpool.tile([K, ngB * P], mybir.dt.bfloat16)
            br_t = cpool.tile([K, wB], mybir.dt.bfloat16)
            out_t = cpool.tile([P, ngA + ngB], mybir.dt.float32)
            nc.sync.dma_start(ar_t[:], ar_d.ap())
            nc.sync.dma_start(al_t[:], al_d.ap())
            nc.sync.dma_start(br_t[:], br_d.ap())
            nc.sync.dma_start(bl_t[:], bl_d.ap())

            def run_pass(l_t, r_t, widths, col0):
                roff = 0
                for g, G in enumerate(widths):
                    wt = G * W
                    ps = pspool.tile([P, KMAX * W], mybir.dt.float32, tag="d")
                    off = 0
                    while off < wt:
                        cw = min(BANK, wt - off)
                        nc.tensor.matmul(
                            ps[:, off:off + cw],
                            l_t[:, g * P:(g + 1) * P],
                            r_t[:, roff + off:roff + off + cw],
                            start=True, stop=True,
                        )
                        off += cw
                    nc.vector.tensor_reduce(
                        out_t[:, col0 + g: col0 + g + 1], ps[:, :wt],
                        axis=mybir.AxisListType.X, op=mybir.AluOpType.max,
                    )
                    roff += wt

            run_pass(al_t, ar_t, widthsA, 0)
            run_pass(bl_t, br_t, widthsB, ngA)
            nc.sync.dma_start(out_d.ap(), out_t[:])

    nc.compile()
    return nc


def _pack(stat_pts, mov_pts, cores, widths):
    """Pack per-core lhsT [K, ng*128] and rhs [K, sum(G)*128] bf16 arrays."""
    ng = len(widths)
    wtot = sum(widths) * W
    l_maps, r_maps = [], []
    for c in range(NCORES):
        l_arr = np.zeros((K, ng * P), dtype=BF16)
        r_arr = np.zeros((K, wtot), dtype=BF16)
        roff = 0
        for g, G in enumerate(widths):
            t, bl = cores[c][g]
            if t is None:
                t, bl = cores[c][0][0], cores[c][0][1]
            st = stat_pts[t * P:(t + 1) * P]
            cshift = st.mean(0)
            sp = st - cshift
            sh, slo = _bf16_pair(sp)
            s2h, s2l = _bf16_pair((sp ** 2).sum(1))
            lblk = np.zeros((K, P), dtype=BF16)
            two_sh = (2.0 * sh.astype(np.float64)).astype(BF16)
            two_sl = (2.0 * slo.astype(np.float64)).astype(BF16)
            lblk[0:3] = two_sh.T
            lblk[3:6] = two_sh.T
            lblk[6:9] = two_sl.T
            lblk[9] = (-s2h.astype(np.float64)).astype(BF16)
            lblk[10] = (-s2l.astype(np.float64)).astype(BF16)
            lblk[11] = BF16(-1.0)
            lblk[12] = BF16(-1.0)
            l_arr[:, g * P:(g + 1) * P] = lblk
            for b in range(G):
                j = bl[b % len(bl)]
                mp = mov_pts[j * W:(j + 1) * W] - cshift
                mh, mlo = _bf16_pair(mp)
                m2h, m2l = _bf16_pair((mp ** 2).sum(1))
                rblk = np.zeros((K, W), dtype=BF16)
                rblk[0:3] = mh.T
                rblk[3:6] = mlo.T
                rblk[6:9] = mh.T
                rblk[9] = BF16(1.0)
                rblk[10] = BF16(1.0)
                rblk[11] = m2h
                rblk[12] = m2l
                r_arr[:, roff + b * W: roff + (b + 1) * W] = rblk
            roff += G * W
        l_maps.append(l_arr)
        r_maps.append(r_arr)
    return l_maps, r_maps


_CACHE = {}


def prepare(x, y):
    """Build (nc, in_maps, structure) for the given full inputs."""
    x = np.asarray(x, np.float64)
    y = np.asarray(y, np.float64)
    st = _build_structure(x, y)
    key = (tuple(st["widthsA"]), tuple(st["widthsB"]))
    if key not in _CACHE:
        _CACHE[key] = build_nc(st["widthsA"], st["widthsB"])
    nc = _CACHE[key]
    alm, arm = _pack(st["xs"], st["ys"], st["coresA"], st["widthsA"])
    blm, brm = _pack(st["ys"], st["xs"], st["coresB"], st["widthsB"])
    in_maps = [{"al": alm[c], "ar": arm[c], "bl": blm[c], "br": brm[c]}
               for c in range(NCORES)]
    return nc, in_maps, st


def kernel(x, y):
    nc, in_maps, st = prepare(x, y)
    res = bass_utils.run_bass_kernel_spmd(nc, in_maps,
                                          core_ids=list(range(NCORES)))

    ngA = len(st["widthsA"])
    d1 = np.full(N, np.inf)
    d2 = np.full(M, np.inf)
    for c in range(NCORES):
        out = res.results[c]["out"].astype(np.float64)
        for g in range(ngA):
            t = st["coresA"][c][g][0]
            if t is None:
                continue
            idx = st["xp"][t * P:(t + 1) * P]
            d1[idx] = np.minimum(d1[idx], -out[:, g])
        for g in range(len(st["widthsB"])):
            t = st["coresB"][c][g][0]
            if t is None:
                continue
            idx = st["yp"][t * P:(t + 1) * P]
            d2[idx] = np.minimum(d2[idx], -out[:, ngA + g])
    val = (np.maximum(d1, 0).sum() + np.maximum(d2, 0).sum()) / (N + M)
    return np.array(val, dtype=np.float32)


if __name__ == "__main__":
    np.random.seed(0)
    x = np.random.randn(N, D).astype(np.float32)
    y = np.random.randn(M, D).astype(np.float32)
    print("kernel:", kernel(x, y))


# revision 9
# speedup vs baseline: 12.0844x; 1.2297x over previous
"""Chamfer distance kernel for Trainium2 (8 NeuronCores, SPMD).

Strategy: pruned nearest-neighbour evaluation.
------------------------------------------------
Both point sets are kd-sorted into 128 spatially compact tiles of 128
points.  On the host (during input prep) rigorous triangle-inequality
bounds select, for every stationary tile, the small set of moving blocks
that can possibly contain a nearest neighbour of any of its points:
  ub(p) = exact min distance from p to the points of its 2 nearest blocks
  lb(p,B) = squared distance from p to block B's AABB
  block B is a candidate for tile T iff any p in T has lb(p,B) <= ub(p).
On this data that keeps ~3.3 candidate blocks per tile (~2.6% of the
dense 16384x16384 distance matrix) while provably containing every true
nearest neighbour, so the result is exact up to arithmetic rounding.

Two passes: pass A (stationary = x tiles) produces the row mins (dist1),
pass B (stationary = y tiles) produces the column mins (dist2) -- no
cross-partition reduction and no inter-core communication is needed;
the host just gathers per-tile min vectors.

SPMD layout: all 8 cores run one NEFF.  Tiles are snake-dealt to cores
by candidate count and sorted within each core, so group g has the same
(max-padded) width on every core; padding duplicates a real candidate
block (harmless under max).  Data differences live entirely in the
per-core input tensors; instruction offsets are structure-only.

Per group the whole stationary tile shares one translation (its own
centroid), so the PE loads one [13,128] stationary operand per group and
streams the group's candidate blocks as 512-wide moving chunks (one
PSUM bank each), computing NEGATED squared distances
  -d = 2x.y - x2 - y2
via a K=13 bf16 contraction (hi/lo-split coordinates keep the products
exact enough for ~1e-4 relative error).  One DVE tensor_reduce(max) per
group then yields -min d for each stationary point directly from PSUM.
"""
import sys

sys.path.insert(0, "/opt/trn_rl_repo")

import numpy as np
import ml_dtypes

import concourse.bass as bass
import concourse.tile as tile
from concourse import bacc, mybir
from concourse import bass_utils

BF16 = ml_dtypes.bfloat16

N = 16384
M = 16384
D = 3
NCORES = 8
P = 128                 # partition dim == stationary tile size
W = 32                  # moving block size
NTILE = N // P          # 128 stationary tiles each side
K = 13                  # augmented contraction depth
KMAX = 16               # max blocks per group (k*W <= 512 = one PSUM bank)
BANK = 512              # fp32 per PSUM bank == max matmul width
NWARM = 2               # warmup groups (4 x N=512 matmuls each)


def _bf16_pair(a):
    hi = a.astype(BF16)
    lo = (a - hi.astype(np.float64)).astype(BF16)
    return hi, lo


def kd_sort(pts, n_tiles):
    groups = [np.arange(len(pts))]
    while len(groups) < n_tiles:
        nxt = []
        for g in groups:
            p = pts[g]
            dim = int(np.argmax(p.max(0) - p.min(0)))
            order = np.argsort(p[:, dim], kind="stable")
            half = len(g) // 2
            nxt.append(g[order[:half]])
            nxt.append(g[order[half:]])
        groups = nxt
    return np.concatenate(groups)


def _candidates(stat, mov):
    """Per stationary 128-tile: candidate moving 128-blocks (rigorous)."""
    nmv = len(mov) // W
    mv = mov.reshape(nmv, W, 3)
    lo, hi = mv.min(1), mv.max(1)
    cm = mv.mean(1)
    d2c = ((stat[:, None, :] - cm[None]) ** 2).sum(-1)
    ub = np.full(len(stat), np.inf)
    near2 = np.argsort(d2c, 1)[:, :2]
    for col in range(2):
        near = near2[:, col]
        for j in range(nmv):
            sel = near == j
            if sel.any():
                d = ((stat[sel][:, None, :] - mv[j][None]) ** 2).sum(-1).min(1)
                ub[sel] = np.minimum(ub[sel], d)
    dx = np.maximum(np.maximum(lo[None] - stat[:, None],
                               stat[:, None] - hi[None]), 0.0)
    lb = (dx ** 2).sum(-1)
    needed = lb <= ub[:, None] * (1 + 1e-9)
    pair = needed.reshape(-1, P, nmv).any(1)
    return [np.flatnonzero(pair[t]).tolist() for t in range(pair.shape[0])]


def _deal(cands):
    """Snake-deal tiles to cores by k desc; split k>KMAX; sort within core.
    Returns per-core group lists [(tile_id, [block ids])] (padded with None
    groups so counts match) plus the shared per-position widths."""
    groups = []
    for t, cl in enumerate(cands):
        for s in range(0, len(cl), KMAX):
            groups.append((t, cl[s:s + KMAX]))
    groups.sort(key=lambda g: -len(g[1]))
    cores = [[] for _ in range(NCORES)]
    for n, g in enumerate(groups):
        c = n % (2 * NCORES)
        c = c if c < NCORES else 2 * NCORES - 1 - c
        cores[c].append(g)
    ng = max(len(c) for c in cores)
    for c in cores:
        while len(c) < ng:
            c.append((None, []))
        c.sort(key=lambda g: -len(g[1]))
    widths = [max(1, max(len(cores[c][g][1]) for c in range(NCORES)))
              for g in range(ng)]
    return cores, widths


def _build_structure(x, y):
    # kd-sort to 32-point leaves; 128-point stationary tiles are the
    # level-7 kd nodes (4 consecutive leaves), so one order serves both
    # the stationary and the moving role of each point set.
    xp = kd_sort(x, N // W)
    yp = kd_sort(y, M // W)
    xs, ys = x[xp], y[yp]
    candA = _candidates(xs, ys)     # x tiles -> y blocks
    candB = _candidates(ys, xs)     # y tiles -> x blocks
    coresA, widthsA = _deal(candA)
    coresB, widthsB = _deal(candB)
    return dict(xp=xp, yp=yp, xs=xs, ys=ys,
                coresA=coresA, widthsA=widthsA,
                coresB=coresB, widthsB=widthsB)


def build_nc(widthsA, widthsB):
    ngA, ngB = len(widthsA), len(widthsB)
    wA = sum(widthsA) * W
    wB = sum(widthsB) * W
    outw = ngA + ngB + NWARM

    nc = bacc.Bacc("TRN2", target_bir_lowering=False, debug=False,
                   num_devices=NCORES)
    al_d = nc.dram_tensor("al", [K, ngA * P], mybir.dt.bfloat16,
                          kind="ExternalInput")
    ar_d = nc.dram_tensor("ar", [K, wA], mybir.dt.bfloat16,
                          kind="ExternalInput")
    bl_d = nc.dram_tensor("bl", [K, ngB * P], mybir.dt.bfloat16,
                          kind="ExternalInput")
    br_d = nc.dram_tensor("br", [K, wB], mybir.dt.bfloat16,
                          kind="ExternalInput")
    out_d = nc.dram_tensor("out", [P, outw], mybir.dt.float32,
                           kind="ExternalOutput")

    with tile.TileContext(nc) as tc:
        with (
            tc.tile_pool(name="const", bufs=1) as cpool,
            tc.tile_pool(name="psw", bufs=1, space="PSUM") as pswarm,
            tc.tile_pool(name="ps", bufs=4, space="PSUM") as pspool,
        ):
            al_t = cpool.tile([K, ngA * P], mybir.dt.bfloat16)
            ar_t = cpool.tile([K, wA], mybir.dt.bfloat16)
            bl_t = cpool.tile([K, ngB * P], mybir.dt.bfloat16)
            br_t = cpool.tile([K, wB], mybir.dt.bfloat16)
            wu_t = cpool.tile([K, 640], mybir.dt.bfloat16)
            out_t = cpool.tile([P, outw], mybir.dt.float32)
            nc.sync.dma_start(al_t[:], al_d.ap())
            nc.sync.dma_start(ar_t[:], ar_d.ap())
            nc.sync.dma_start(bl_t[:], bl_d.ap())
            nc.sync.dma_start(br_t[:], br_d.ap())

            # Warmup: keep the PE busy during the input DMAs so the HAM
            # clock gate reaches 8/8 before the real matmuls start.
            nc.vector.memset(wu_t[:], 0.0)
            for wg in range(NWARM):
                psw = pswarm.tile([P, 4 * BANK], mybir.dt.float32, tag="w")
                for i in range(4):
                    nc.tensor.matmul(psw[:, i * BANK:(i + 1) * BANK],
                                     wu_t[:, 0:P], wu_t[:, P:P + BANK],
                                     start=True, stop=True)
                nc.vector.tensor_reduce(
                    out_t[:, ngA + ngB + wg: ngA + ngB + wg + 1], psw[:],
                    axis=mybir.AxisListType.X, op=mybir.AluOpType.max,
                )

            def run_pass(l_t, r_t, widths, col0):
                roff = 0
                for g, G in enumerate(widths):
                    wt = G * W
                    ps = pspool.tile([P, BANK], mybir.dt.float32, tag="d")
                    nc.tensor.matmul(
                        ps[:, 0:wt],
                        l_t[:, g * P:(g + 1) * P],
                        r_t[:, roff:roff + wt],
                        start=True, stop=True,
                    )
                    nc.vector.tensor_reduce(
                        out_t[:, col0 + g: col0 + g + 1], ps[:, :wt],
                        axis=mybir.AxisListType.X, op=mybir.AluOpType.max,
                    )
                    roff += wt

            run_pass(al_t, ar_t, widthsA, 0)
            run_pass(bl_t, br_t, widthsB, ngA)
            nc.sync.dma_start(out_d.ap(), out_t[:])

    nc.compile()
    return nc


def _pack(stat_pts, mov_pts, cores, widths):
    """Pack per-core lhsT [K, ng*128] and rhs [K, sum(G)*128] bf16 arrays."""
    ng = len(widths)
    wtot = sum(widths) * W
    l_maps, r_maps = [], []
    for c in range(NCORES):
        l_arr = np.zeros((K, ng * P), dtype=BF16)
        r_arr = np.zeros((K, wtot), dtype=BF16)
        roff = 0
        for g, G in enumerate(widths):
            t, bl = cores[c][g]
            if t is None:
                t, bl = cores[c][0][0], cores[c][0][1]
            st = stat_pts[t * P:(t + 1) * P]
            cshift = st.mean(0)
            sp = st - cshift
            sh, slo = _bf16_pair(sp)
            s2h, s2l = _bf16_pair((sp ** 2).sum(1))
            lblk = np.zeros((K, P), dtype=BF16)
            two_sh = (2.0 * sh.astype(np.float64)).astype(BF16)
            two_sl = (2.0 * slo.astype(np.float64)).astype(BF16)
            lblk[0:3] = two_sh.T
            lblk[3:6] = two_sh.T
            lblk[6:9] = two_sl.T
            lblk[9] = (-s2h.astype(np.float64)).astype(BF16)
            lblk[10] = (-s2l.astype(np.float64)).astype(BF16)
            lblk[11] = BF16(-1.0)
            lblk[12] = BF16(-1.0)
            l_arr[:, g * P:(g + 1) * P] = lblk
            for b in range(G):
                j = bl[b % len(bl)]
                mp = mov_pts[j * W:(j + 1) * W] - cshift
                mh, mlo = _bf16_pair(mp)
                m2h, m2l = _bf16_pair((mp ** 2).sum(1))
                rblk = np.zeros((K, W), dtype=BF16)
                rblk[0:3] = mh.T
                rblk[3:6] = mlo.T
                rblk[6:9] = mh.T
                rblk[9] = BF16(1.0)
                rblk[10] = BF16(1.0)
                rblk[11] = m2h
                rblk[12] = m2l
                r_arr[:, roff + b * W: roff + (b + 1) * W] = rblk
            roff += G * W
        l_maps.append(l_arr)
        r_maps.append(r_arr)
    return l_maps, r_maps


_CACHE = {}


def prepare(x, y):
    """Build (nc, in_maps, structure) for the given full inputs."""
    x = np.asarray(x, np.float64)
    y = np.asarray(y, np.float64)
    st = _build_structure(x, y)
    key = (tuple(st["widthsA"]), tuple(st["widthsB"]))
    if key not in _CACHE:
        _CACHE[key] = build_nc(st["widthsA"], st["widthsB"])
    nc = _CACHE[key]
    alm, arm = _pack(st["xs"], st["ys"], st["coresA"], st["widthsA"])
    blm, brm = _pack(st["ys"], st["xs"], st["coresB"], st["widthsB"])
    in_maps = [{"al": alm[c], "ar": arm[c], "bl": blm[c], "br": brm[c]}
               for c in range(NCORES)]
    return nc, in_maps, st


def kernel(x, y):
    nc, in_maps, st = prepare(x, y)
    res = bass_utils.run_bass_kernel_spmd(nc, in_maps,
                                          core_ids=list(range(NCORES)))

    ngA = len(st["widthsA"])
    d1 = np.full(N, np.inf)
    d2 = np.full(M, np.inf)
    for c in range(NCORES):
        out = res.results[c]["out"].astype(np.float64)
        for g in range(ngA):
            t = st["coresA"][c][g][0]
            if t is None:
                continue
            idx = st["xp"][t * P:(t + 1) * P]
            d1[idx] = np.minimum(d1[idx], -out[:, g])
        for g in range(len(st["widthsB"])):
            t = st["coresB"][c][g][0]
            if t is None:
                continue
            idx = st["yp"][t * P:(t + 1) * P]
            d2[idx] = np.minimum(d2[idx], -out[:, ngA + g])
    val = (np.maximum(d1, 0).sum() + np.maximum(d2, 0).sum()) / (N + M)
    return np.array(val, dtype=np.float32)


if __name__ == "__main__":
    np.random.seed(0)
    x = np.random.randn(N, D).astype(np.float32)
    y = np.random.randn(M, D).astype(np.float32)
    print("kernel:", kernel(x, y))
